# revision 57
# baseline (speedup 1.0000x reference)
"""Trainium2 Bass kernel for nn_AlignmentModel (N=512, M=512, D=768).

score[n,m] = sum_d text[n,d] * tanh( (text@W2)[n,d] + (visual@W3^T)[m,d] * C[n,m] )
C = tanh((text@W1) @ visual^T)

Sharded over N across 8 cores (64 rows each); visual + weights replicated.
Each core computes its 64 rows of score; host concatenates.

Per-core layout: d on partitions (6 chunks of 128), m on free dim.
  arg[d,m] = G[d,m]*C[n,m] + W2T[d,n]   (G = (visual@W3^T)^T, shape (768, 512))
  - C-row broadcast across partitions: DMA with stride-0 partition AP
  - multiply: DVE tensor_tensor for chunks 0-3, GPSIMD for chunks 4-5
  - +W2T and tanh: fused in ACT activation bias (per-partition column)
  - sum_d text[n,d]*tanh(arg): PE matvec per d-chunk (fp32r), PSUM-accumulated
Steady state is ACT-bound: 6 tanh instrs per n ~= 3.67us; DVE/GPSIMD/PE overlap
underneath.
"""

import contextlib
import sys

import numpy as np

sys.path.insert(0, "/opt/trn_rl_repo")

N_CORES = 8
N_FULL, M, D = 512, 512, 768
N = N_FULL // N_CORES  # 64 rows per core
DC = D // 128  # 6 d-chunks
VC = M // 128  # 4 visual row chunks

_CACHED_NC = None


def _build():
    import concourse.bass as bass
    import concourse.tile as tile
    from concourse import bacc, mybir
    from concourse.masks import make_identity

    F32 = mybir.dt.float32
    F32R = mybir.dt.float32r
    Tanh = mybir.ActivationFunctionType.Tanh

    nc = bacc.Bacc("TRN2", target_bir_lowering=False, debug=False)

    text = nc.dram_tensor("text", [N, D], F32, kind="ExternalInput").ap()
    visual = nc.dram_tensor("visual", [M, D], F32, kind="ExternalInput").ap()
    W1 = nc.dram_tensor("W1", [D, D], F32, kind="ExternalInput").ap()
    W2 = nc.dram_tensor("W2", [D, D], F32, kind="ExternalInput").ap()
    W3 = nc.dram_tensor("W3", [D, D], F32, kind="ExternalInput").ap()
    score = nc.dram_tensor("score", [N, M], F32, kind="ExternalOutput").ap()

    with tile.TileContext(nc) as tc, contextlib.ExitStack() as octx:
        persist = octx.enter_context(tc.tile_pool(name="persist", bufs=1))
        dram_pool = octx.enter_context(
            tc.tile_pool(name="cdram", bufs=1, space="DRAM")
        )
        C_dram = dram_pool.tile([N, M], F32)
        ident = persist.tile([128, 128], F32)
        make_identity(nc, ident[:])
        ones_sb = persist.tile([1, 128], F32)
        nc.vector.memset(ones_sb[:], 1.0)
        G_sb = persist.tile([128, DC, M], F32)      # G[d,m] as [p, dc, m]
        W2T_sb = persist.tile([128, DC, N], F32)    # (text@W2)^T as [p, dc, n]
        textT_sb = persist.tile([128, DC, N], F32)  # text^T as [p, dc, n]
        textT_r = persist.tile([128, DC, N], F32R)  # fp32r shadow for matvec lhsT
        C_sb = persist.tile([N, M], F32)

        # main-loop pools allocated BEFORE the prologue pool so their SBUF
        # does not overlap (and thus wait on) late prologue readers
        bc_pool = octx.enter_context(tc.tile_pool(name="bc", bufs=5))
        tmp_pool = octx.enter_context(tc.tile_pool(name="tmp", bufs=11))
        t_pool = octx.enter_context(tc.tile_pool(name="tout", bufs=11))
        stage_pool = octx.enter_context(tc.tile_pool(name="stage", bufs=2))
        spsum = octx.enter_context(tc.tile_pool(name="spsum", bufs=2, space="PSUM"))

        with contextlib.ExitStack() as pctx:
            pro = pctx.enter_context(tc.tile_pool(name="prologue", bufs=1))
            ppsum = pctx.enter_context(
                tc.tile_pool(name="ppsum", bufs=4, space="PSUM")
            )
            ppsum_w = pctx.enter_context(
                tc.tile_pool(name="ppsum_w", bufs=2, space="PSUM")
            )

            W1_sb = pro.tile([128, DC, D], F32)
            W2_sb = pro.tile([128, DC, D], F32)
            W3_sb = pro.tile([128, DC, D], F32)
            vis_sb = pro.tile([128, VC, D], F32)
            text_sb = pro.tile([N, D], F32)
            visT_sb = pro.tile([128, DC, M], F32)   # visual^T as [p, dc, m]
            visT_r = pro.tile([128, DC, M], F32R)   # fp32r shadow for G matmul
            W3T_sb = pro.tile([128, DC, D], F32R)   # W3^T as [p, kc, d] (fp32r)
            T1T_sb = pro.tile([128, DC, N], F32)    # (text@W1)^T as [p, dc, n]

            # loads: vis/text first on the scalar HWDGE queue (the ACT
            # sequencer issues them immediately, before any ACT compute);
            # the weight matrices go on the sync queue
            nc.scalar.dma_start(text_sb[:], text)
            for c in range(VC):
                nc.scalar.dma_start(vis_sb[:, c, :], visual[c * 128:(c + 1) * 128, :])
            for c in range(DC):
                nc.sync.dma_start(W1_sb[:, c, :], W1[c * 128:(c + 1) * 128, :])
            for c in range(DC):
                nc.sync.dma_start(W3_sb[:, c, :], W3[c * 128:(c + 1) * 128, :])
            for c in range(DC):
                nc.sync.dma_start(W2_sb[:, c, :], W2[c * 128:(c + 1) * 128, :])

            # text^T: textT[p, dc, n] = text[n, dc*128+p]
            for dc in range(DC):
                pt = ppsum.tile([128, N], F32, tag="pp")
                nc.tensor.transpose(
                    pt[:], text_sb[:, dc * 128:(dc + 1) * 128], ident[:N, :N]
                )
                nc.vector.tensor_copy(textT_sb[:, dc, :], pt[:])
                nc.scalar.copy(textT_r[:, dc, :], pt[:])

            # visual^T via PE transpose: visT[p, dc, m] = visual[m, dc*128+p]
            for dc in range(DC):
                pt = ppsum.tile([128, M], F32, tag="pp")
                for vc in range(VC):
                    nc.tensor.transpose(
                        pt[:, vc * 128:(vc + 1) * 128],
                        vis_sb[:, vc, dc * 128:(dc + 1) * 128],
                        ident[:],
                    )
                nc.vector.tensor_copy(visT_sb[:, dc, :], pt[:])
                nc.scalar.copy(visT_r[:, dc, :], pt[:])

            # T1T[d,n] = sum_k W1[k,d] * textT[k,n]  (= (text@W1)^T), full fp32
            for dc in range(DC):
                pt1 = ppsum.tile([128, N], F32, tag="pp")
                for kc in range(DC):
                    nc.tensor.matmul(
                        pt1[:],
                        W1_sb[:, kc, dc * 128:(dc + 1) * 128],
                        textT_sb[:, kc, :],
                        start=(kc == 0),
                        stop=(kc == DC - 1),
                    )
                nc.vector.tensor_copy(T1T_sb[:, dc, :], pt1[:])

            # W3^T: W3T[p, kc, d] = W3[d, kc*128+p] (half-wide psum tiles
            # so the pool fits in single banks and pipelines)
            for kc in range(DC):
                for h in range(2):
                    pt = ppsum_w.tile([128, D // 2], F32, tag="ppw")
                    for c3 in range(3):
                        c = h * 3 + c3
                        nc.tensor.transpose(
                            pt[:, c3 * 128:(c3 + 1) * 128],
                            W3_sb[:, c, kc * 128:(kc + 1) * 128],
                            ident[:],
                        )
                    nc.vector.tensor_copy(
                        W3T_sb[:, kc, h * (D // 2):(h + 1) * (D // 2)], pt[:]
                    )

            # C = tanh(T1 @ visual^T): psum_c[n,m] = sum_d T1T[d,n]*visT[d,m]
            pc = ppsum.tile([N, M], F32, tag="pp")
            for dc in range(DC):
                nc.tensor.matmul(
                    pc[:],
                    T1T_sb[:, dc, :],
                    visT_sb[:, dc, :],
                    start=(dc == 0),
                    stop=(dc == DC - 1),
                )
            nc.scalar.activation(C_sb[:], pc[:], Tanh)
            nc.sync.dma_start(C_dram[:], C_sb[:])

            # G[d,m] = sum_k W3T[k,d]*visT[k,m] and W2T[d,n], interleaved so
            # each tanh's inputs arrive together; G chunks 4-5 (GPSIMD's
            # multiply inputs, the ramp tail) are hoisted before the last
            # two W2T groups
            def g_group(dc):
                pg = ppsum.tile([128, M], F32, tag="pp")
                for kc in range(DC):
                    nc.tensor.matmul(
                        pg[:],
                        W3T_sb[:, kc, dc * 128:(dc + 1) * 128],
                        visT_r[:, kc, :],
                        start=(kc == 0),
                        stop=(kc == DC - 1),
                    )
                if dc % 2 == 0:
                    nc.vector.tensor_copy(G_sb[:, dc, :], pg[:])
                else:
                    nc.scalar.copy(G_sb[:, dc, :], pg[:])

            def w2t_group(dc):
                pt2 = ppsum.tile([128, N], F32, tag="pp")
                for kc in range(DC):
                    nc.tensor.matmul(
                        pt2[:],
                        W2_sb[:, kc, dc * 128:(dc + 1) * 128],
                        textT_sb[:, kc, :],
                        start=(kc == 0),
                        stop=(kc == DC - 1),
                    )
                nc.scalar.copy(W2T_sb[:, dc, :], pt2[:])

            for step in ["g0", "w0", "g1", "g2", "w1", "g3", "g4", "w2",
                         "g5", "w3", "w4", "w5"]:
                kind, dc = step[0], int(step[1])
                if kind == "g":
                    g_group(dc)
                else:
                    w2t_group(dc)

        # ---- main loop ----
        stage = None
        for n in range(N):
            j = n % 4
            if j == 0:
                stage = stage_pool.tile([1, 4, M], F32)
            psum_s = spsum.tile([1, M], F32)

            # broadcast C[n, :] to 128 partitions via stride-0 DMA from DRAM
            bc_sb = bc_pool.tile([128, M], F32)
            crow = C_dram[n:n + 1, :]
            bc_bcast = bass.AP(
                tensor=crow.tensor, offset=crow.offset,
                ap=[[0, 128]] + crow.ap[1:],
            )
            nc.sync.dma_start(bc_sb[:], bc_bcast)
            bc = bc_sb[:]

            # tmp_dc = G[:, dc, :] * C[n, :]; DVE chunks 0-3, GPSIMD 4-5
            tmps = []
            for dc in range(DC):
                tmp = tmp_pool.tile([128, M], F32, tag="tmp")
                tmps.append(tmp)
                if dc < 4:
                    nc.vector.tensor_mul(tmp[:], G_sb[:, dc, :], bc)
                else:
                    nc.gpsimd.tensor_mul(tmp[:], G_sb[:, dc, :], bc)

            # T = tanh(tmp + W2T[:, n]) fused bias; then matvec accumulate
            for dc in range(DC):
                T_t = t_pool.tile([128, M], F32R, tag="T")
                nc.scalar.activation(
                    T_t[:],
                    tmps[dc][:],
                    Tanh,
                    bias=W2T_sb[:, dc, n:n + 1],
                )
                nc.tensor.matmul(
                    psum_s[:],
                    textT_r[:, dc, n:n + 1],
                    T_t[:],
                    start=(dc == 0),
                    stop=(dc == DC - 1),
                )

            nc.vector.tensor_copy(stage[:, j, :], psum_s[:])
            if j == 3:
                nc.sync.dma_start(
                    score[n - 3:n + 1, :].rearrange("(a b) m -> a b m", a=1),
                    stage[:],
                )

    nc.compile()
    return nc


def _get_nc():
    global _CACHED_NC
    if _CACHED_NC is None:
        _CACHED_NC = _build()
    return _CACHED_NC


def kernel(text_output, visual_output, W1, W2, W3):
    from concourse.bass_utils import run_bass_kernel_spmd

    text_output = np.ascontiguousarray(np.asarray(text_output, dtype=np.float32))
    visual_output = np.ascontiguousarray(np.asarray(visual_output, dtype=np.float32))
    W1 = np.ascontiguousarray(np.asarray(W1, dtype=np.float32))
    W2 = np.ascontiguousarray(np.asarray(W2, dtype=np.float32))
    W3 = np.ascontiguousarray(np.asarray(W3, dtype=np.float32))

    nc = _get_nc()
    in_maps = [
        {
            "text": text_output[i * N:(i + 1) * N],
            "visual": visual_output,
            "W1": W1,
            "W2": W2,
            "W3": W3,
        }
        for i in range(N_CORES)
    ]
    res = run_bass_kernel_spmd(nc, in_maps, core_ids=list(range(N_CORES)))
    return np.concatenate(
        [res.results[i]["score"] for i in range(N_CORES)], axis=0
    )
